# revision 10
# baseline (speedup 1.0000x reference)
"""Trainium2 Bass kernel for nn_EnhancedGraphConv (gnn_message_passing).

Strategy (8 cores): shard the B*N=1280 graph rows as 160 rows/core (cores
0-3 -> batch 0, 4-7 -> batch 1).  The host converts the dense adjacency to
padded neighbor lists (max degree 51 -> D=52 slots/row) and gathers the
neighbor features (x_j) and edge features directly into feature-major bf16
HBM layouts, so the device streams contiguous tiles: no device-side gather
and no input transposes.  All matmuls run in bf16 (1 cycle/row on PE vs 4
for fp32).  The attention softmax runs token-major: scores land in PSUM as
[128 tokens, 13 cols] whose partition p maps to row p%32, so row max / sum
use small cross-partition DVE ops, and the weighted message reduction is a
PE matmul against a w-scaled block-identity selection matrix.  Exp (softmax)
and Sigmoid (gates) live in different ACT function sets, so all sigmoid-set
work happens in phase 1 and a single batched exp runs in phase 2: exactly
two ACT table loads per invocation.
"""
import numpy as np
from contextlib import ExitStack

import concourse.bass as bass
import concourse.bacc as bacc
import concourse.tile as tile
from concourse import mybir
from concourse.bass_utils import run_bass_kernel_spmd

F32 = mybir.dt.float32
BF16 = mybir.dt.bfloat16
AF = mybir.ActivationFunctionType
OP = mybir.AluOpType

B, N, C, O, E = 2, 640, 64, 64, 18
D = 52            # neighbor slots per row (max degree 51)
RG = 32           # rows per group
NCORES = 8
RPC = (B * N) // NCORES   # 160 rows per core
NG = RPC // RG            # 5 groups
TG = D * RG               # 1664 tokens per group
CH = 416                  # matmul moving chunk
NCH = TG // CH            # 4 chunks per group
NCOL = TG // 128          # 13 token-major columns per group


def _build_nc(debug=False, reps=1):
    nc = bacc.Bacc("TRN2", target_bir_lowering=False)
    t = {}
    winp = [
        ("We1", [E, 64]), ("We2", [64, 64]), ("We3", [64, 32]),
        ("W96", [96, 128]), ("Wxi", [64, 128]), ("Wn", [64, 64]),
        ("W22", [128, 128]), ("Wa3", [32, 1]), ("Ws", [64, 64]),
        ("Wc1", [128, 64]), ("Wc2", [64, 64]),
        ("bsel", [128, RG]), ("identb", [64, 64]),
    ]
    binp = [
        ("be1", [64, 1]), ("be2", [64, 1]), ("be3", [32, 1]),
        ("bhg", [128, 1]), ("bn", [64, 1]), ("ba2", [32, 1]),
        ("bg2", [64, 1]), ("bs", [64, 1]), ("bc1", [64, 1]), ("bc2", [64, 1]),
    ]
    for name, shape in winp:
        t[name] = nc.dram_tensor(name, shape, BF16, kind="ExternalInput")
    for name, shape in binp:
        t[name] = nc.dram_tensor(name, shape, F32, kind="ExternalInput")
    t["identf"] = nc.dram_tensor("identf", [64, 64], F32, kind="ExternalInput")
    t["xjf"] = nc.dram_tensor("xjf", [64, NG * TG], BF16, kind="ExternalInput")
    t["eff"] = nc.dram_tensor("eff", [E, NG * TG], BF16, kind="ExternalInput")
    t["xrf"] = nc.dram_tensor("xrf", [64, RPC], BF16, kind="ExternalInput")
    t["amt"] = nc.dram_tensor("amt", [128, NG * NCOL], F32, kind="ExternalInput")
    t["out"] = nc.dram_tensor("out", [RPC, O], F32, kind="ExternalOutput")
    if debug:
        t["dbg_h2"] = nc.dram_tensor("dbg_h2", [32, TG], F32, kind="ExternalOutput")
        t["dbg_gtn"] = nc.dram_tensor("dbg_gtn", [64, TG], F32, kind="ExternalOutput")
        t["dbg_wexp"] = nc.dram_tensor("dbg_wexp", [128, NG * NCOL], F32, kind="ExternalOutput")
        t["dbg_msg"] = nc.dram_tensor("dbg_msg", [RG, 64], F32, kind="ExternalOutput")

    with tile.TileContext(nc) as tc, ExitStack() as ctx:
        w = ctx.enter_context(tc.tile_pool(name="w", bufs=1))
        big = ctx.enter_context(tc.tile_pool(name="big", bufs=2))
        sm = ctx.enter_context(tc.tile_pool(name="sm", bufs=2))
        mm = ctx.enter_context(tc.tile_pool(name="mm", bufs=3, space="PSUM"))
        pp = ctx.enter_context(tc.tile_pool(name="pp", bufs=2, space="PSUM"))
        pq = ctx.enter_context(tc.tile_pool(name="pq", bufs=1, space="PSUM"))

        # ---- weights / constants -> SBUF
        wt = {}
        for name, shape in winp:
            wt[name] = w.tile(shape, BF16, name=name)
        for name, shape in binp:
            wt[name] = w.tile(shape, F32, name=name)
        for name, _ in winp + binp:
            nc.sync.dma_start(out=wt[name][:], in_=t[name][:])
        identf = w.tile([64, 64], F32, name="identf")
        nc.sync.dma_start(out=identf[:], in_=t["identf"][:])
        xrf = w.tile([64, RPC], BF16, name="xrf")
        nc.sync.dma_start(out=xrf[:], in_=t["xrf"][:])
        amt = w.tile([128, NG * NCOL], F32, name="amt")
        nc.sync.dma_start(out=amt[:], in_=t["amt"][:])

        # ---- prologue: self features for this core's rows
        pself = pp.tile([64, RPC], F32, name="pp")
        nc.tensor.matmul(pself[:], wt["Ws"][:], xrf[:], start=True, stop=True)
        selff = w.tile([64, RPC], BF16, name="selff")
        nc.scalar.activation(selff[:], pself[:], AF.Identity, bias=wt["bs"][:])

        # persistent per-rep state
        smg = w.tile([128, NG * NCOL], F32, name="smg")
        gtm = w.tile([128, NG * NCOL * 64], BF16, name="gtm")

        for rep in range(reps):
            # ================= phase 1: per-group MLPs (sigmoid ACT set)
            for g in range(NG):
                gsl = slice(g * TG, (g + 1) * TG)
                xs = big.tile([128, TG], BF16, name="xs")
                nc.sync.dma_start(out=xs[:64, :], in_=t["xjf"][:, gsl])
                ef = big.tile([E, TG], BF16, name="ef")
                nc.sync.dma_start(out=ef[:], in_=t["eff"][:, gsl])

                pe1 = big.tile([64, TG], BF16, name="pe1")
                pe2 = big.tile([64, TG], BF16, name="pe2")
                for q in range(NCH):
                    cols = slice(q * CH, (q + 1) * CH)
                    ps = mm.tile([128, CH], F32, name="mm")
                    nc.tensor.matmul(ps[:64, :], wt["We1"][:], ef[:, cols],
                                     start=True, stop=True)
                    nc.scalar.activation(pe1[:, cols], ps[:64, :], AF.Relu,
                                         bias=wt["be1"][:])
                for q in range(NCH):
                    cols = slice(q * CH, (q + 1) * CH)
                    ps = mm.tile([128, CH], F32, name="mm")
                    nc.tensor.matmul(ps[:64, :], wt["We2"][:], pe1[:, cols],
                                     start=True, stop=True)
                    nc.scalar.activation(pe2[:, cols], ps[:64, :], AF.Relu,
                                         bias=wt["be2"][:])
                for q in range(NCH):
                    cols = slice(q * CH, (q + 1) * CH)
                    ps = mm.tile([128, CH], F32, name="mm")
                    nc.tensor.matmul(ps[:32, :], wt["We3"][:], pe2[:, cols],
                                     start=True, stop=True)
                    nc.vector.tensor_scalar(out=xs[64:96, cols], in0=ps[:32, :],
                                            scalar1=wt["be3"][:], scalar2=0.0,
                                            op0=OP.add, op1=OP.max)
                # hg = relu(W96.[xj;pe3] + Wxi.xi + bhg)
                hg = big.tile([128, TG], BF16, name="hg")
                for q in range(NCH):
                    cols = slice(q * CH, (q + 1) * CH)
                    ps = mm.tile([128, CH], F32, name="mm")
                    nc.tensor.matmul(ps[:], wt["W96"][:], xs[:96, cols],
                                     start=True, stop=False)
                    xi_b = bass.AP(
                        tensor=xrf.tensor,
                        offset=xrf[:, g * RG:(g + 1) * RG].offset,
                        ap=[xrf[:].ap[0], [0, CH // RG], [1, RG]])
                    nc.tensor.matmul(ps[:], wt["Wxi"][:], xi_b,
                                     start=False, stop=True)
                    nc.scalar.activation(hg[:, cols], ps[:], AF.Relu,
                                         bias=wt["bhg"][:])
                tn = big.tile([64, TG], BF16, name="tn")
                for q in range(NCH):
                    cols = slice(q * CH, (q + 1) * CH)
                    ps = mm.tile([128, CH], F32, name="mm")
                    nc.tensor.matmul(ps[:64, :], wt["Wn"][:], xs[:64, cols],
                                     start=True, stop=True)
                    nc.vector.tensor_scalar_add(out=tn[:, cols],
                                                in0=ps[:64, :],
                                                scalar1=wt["bn"][:])
                h2 = big.tile([32, TG], BF16, name="h2")
                gates = big.tile([64, TG], BF16, name="gates")
                for q in range(NCH):
                    cols = slice(q * CH, (q + 1) * CH)
                    ps = mm.tile([128, CH], F32, name="mm")
                    nc.tensor.matmul(ps[:], wt["W22"][:], hg[:, cols],
                                     start=True, stop=True)
                    nc.vector.tensor_scalar(out=h2[:, cols], in0=ps[:32, :],
                                            scalar1=wt["ba2"][:], scalar2=0.0,
                                            op0=OP.add, op1=OP.max)
                    nc.scalar.activation(gates[:, cols], ps[64:128, :],
                                         AF.Sigmoid, bias=wt["bg2"][:])
                gtn = big.tile([64, TG], BF16, name="gtn")
                nc.vector.tensor_tensor(out=gtn[:], in0=gates[:], in1=tn[:],
                                        op=OP.mult)
                # scores -> masked, token-major [128, NCOL]
                psc = pq.tile([128, NCOL], F32, name="pq")
                for c in range(NCOL):
                    nc.tensor.matmul(psc[:, c:c + 1],
                                     h2[:, c * 128:(c + 1) * 128],
                                     wt["Wa3"][:], start=True, stop=True)
                csl = slice(g * NCOL, (g + 1) * NCOL)
                nc.vector.tensor_tensor(out=smg[:, csl], in0=psc[:],
                                        in1=amt[:, csl], op=OP.add)
                # transpose gtn to token-major chunks [128, 64]
                for c in range(NCOL):
                    ptm = pp.tile([128, 64], BF16, name="ppb")
                    nc.tensor.transpose(ptm[:], gtn[:, c * 128:(c + 1) * 128],
                                        wt["identb"][:])
                    k = (g * NCOL + c) * 64
                    nc.scalar.activation(gtm[:, k:k + 64], ptm[:], AF.Copy)
                if debug and g == 0 and rep == 0:
                    nc.sync.dma_start(out=t["dbg_h2"][:], in_=h2[:])
                    nc.sync.dma_start(out=t["dbg_gtn"][:], in_=gtn[:])

            # ================= phase 2: batched masked softmax pieces (exp set)
            # cross-partition folds route one operand through PSUM (walrus
            # forbids SB+SB TensorTensor with mismatched base partitions)
            rmax = sm.tile([128, NG], F32, name="rmax")
            nc.vector.tensor_reduce(
                out=rmax[:], in_=smg[:].rearrange("p (g c) -> p g c", g=NG),
                axis=mybir.AxisListType.X, op=OP.max)
            prmax = pp.tile([128, NG], F32, name="pp")
            nc.vector.tensor_copy(out=prmax[:], in_=rmax[:])
            m2 = sm.tile([64, NG], F32, name="m2")
            nc.vector.tensor_tensor(out=m2[:], in0=prmax[64:128, :],
                                    in1=rmax[0:64, :], op=OP.max)
            pm2 = pp.tile([64, NG], F32, name="pp")
            nc.vector.tensor_copy(out=pm2[:], in_=m2[:])
            nm = sm.tile([32, NG], F32, name="nm")
            nc.vector.tensor_tensor(out=nm[:], in0=pm2[32:64, :],
                                    in1=m2[0:32, :], op=OP.max)
            nc.vector.tensor_scalar_mul(out=nm[:], in0=nm[:], scalar1=-1.0)
            pnm = pp.tile([32, NG], F32, name="pp")
            nc.vector.tensor_copy(out=pnm[:], in_=nm[:])
            nm128 = sm.tile([128, NG], F32, name="nm128")
            for blk in range(4):
                nc.vector.tensor_copy(out=nm128[blk * 32:(blk + 1) * 32, :],
                                      in_=pnm[:])
            wsub = sm.tile([128, NG * NCOL], F32, name="wsub")
            nm_b = bass.AP(tensor=nm128.tensor, offset=nm128[:].offset,
                           ap=[nm128[:].ap[0], [1, NG], [0, NCOL]])
            nc.vector.tensor_tensor(out=wsub[:], in0=smg[:], in1=nm_b,
                                    op=OP.add)
            wexp = sm.tile([128, NG * NCOL], BF16, name="wexp")
            nc.scalar.activation(wexp[:], wsub[:], AF.Exp)
            if debug and rep == 0:
                nc.sync.dma_start(out=t["dbg_wexp"][:], in_=wexp[:])
            # Z per (row, group) via PE: block-identity contracts partitions
            pZ = pp.tile([32, NG * NCOL], F32, name="pp")
            nc.tensor.matmul(pZ[:], wt["bsel"][:], wexp[:],
                             start=True, stop=True)
            invz = sm.tile([32, NG], F32, name="invz")
            nc.vector.tensor_reduce(
                out=invz[:], in_=pZ[:].rearrange("p (g c) -> p g c", g=NG),
                axis=mybir.AxisListType.X, op=OP.add)
            nc.vector.tensor_scalar_add(out=invz[:], in0=invz[:], scalar1=1e-30)
            nc.vector.reciprocal(out=invz[:], in_=invz[:])

            # ================= phase 3: weighted reduce + output MLP (exp set)
            for g in range(NG):
                wsel = sm.tile([128, NCOL * RG], BF16, name="wsel")
                bsel_b = bass.AP(tensor=wt["bsel"].tensor,
                                 offset=wt["bsel"][:].offset,
                                 ap=[wt["bsel"][:].ap[0], [0, NCOL], [1, RG]])
                we_b = bass.AP(tensor=wexp.tensor,
                               offset=wexp[:, g * NCOL:(g + 1) * NCOL].offset,
                               ap=[wexp[:].ap[0], [1, NCOL], [0, RG]])
                nc.vector.tensor_tensor(out=wsel[:], in0=bsel_b, in1=we_b,
                                        op=OP.mult)
                pmsg = pq.tile([32, 64], F32, name="pq")
                for c in range(NCOL):
                    k = (g * NCOL + c) * 64
                    nc.tensor.matmul(pmsg[:], wsel[:, c * RG:(c + 1) * RG],
                                     gtm[:, k:k + 64],
                                     start=(c == 0), stop=(c == NCOL - 1))
                msg = sm.tile([32, 64], BF16, name="msg")
                nc.vector.tensor_scalar_mul(out=msg[:], in0=pmsg[:],
                                            scalar1=invz[:, g:g + 1])
                if debug and g == 0 and rep == 0:
                    nc.sync.dma_start(out=t["dbg_msg"][:], in_=msg[:])
                pmt = pp.tile([64, RG], BF16, name="ppb")
                nc.tensor.transpose(pmt[:], msg[:], wt["identb"][:32, :32])
                comb = sm.tile([128, RG], BF16, name="comb")
                nc.scalar.activation(comb[:64, :],
                                     selff[:, g * RG:(g + 1) * RG], AF.Copy)
                nc.scalar.activation(comb[64:128, :], pmt[:], AF.Copy)
                pc1 = pp.tile([64, RG], F32, name="pp")
                nc.tensor.matmul(pc1[:], wt["Wc1"][:], comb[:],
                                 start=True, stop=True)
                c1 = sm.tile([64, RG], BF16, name="c1")
                nc.scalar.activation(c1[:], pc1[:], AF.Relu, bias=wt["bc1"][:])
                pc2 = pp.tile([64, RG], F32, name="pp")
                nc.tensor.matmul(pc2[:], wt["Wc2"][:], c1[:],
                                 start=True, stop=True)
                ofm = sm.tile([64, RG], F32, name="ofm")
                nc.scalar.activation(ofm[:], pc2[:], AF.Identity,
                                     bias=wt["bc2"][:])
                por = pp.tile([RG, 64], F32, name="pp")
                nc.tensor.transpose(por[:], ofm[:], identf[:])
                orow = sm.tile([RG, 64], F32, name="orow")
                nc.vector.tensor_copy(out=orow[:], in_=por[:])
                nc.sync.dma_start(out=t["out"][g * RG:(g + 1) * RG, :],
                                  in_=orow[:])
    nc.compile()
    return nc


_NC = None


def _host_prep(x, adjacency, edge_features, weights):
    """Build per-core input maps (bf16 feature-major gathered layouts)."""
    from ml_dtypes import bfloat16
    adj = adjacency > 0
    order = np.argsort(~adj, axis=-1, kind="stable")   # [B, N, N]
    deg = adj.sum(-1)                                  # [B, N]
    assert deg.max() <= D, f"degree {deg.max()} exceeds {D} slots"
    jidx = order[:, :, :D].astype(np.int64)            # [B, N, D]
    slot = np.arange(D)[None, None, :]
    valid = slot < deg[:, :, None]                     # [B, N, D]
    jidx = np.where(valid, jidx, 0)

    C2 = 2 * C
    Wa1, Wg1 = weights["Wa1"], weights["Wg1"]
    W96 = np.concatenate([
        np.concatenate([Wa1[C:C2], Wg1[:C]], 1),       # xj rows  [64, 128]
        np.concatenate([Wa1[C2:], Wg1[C:]], 1),        # pe rows  [32, 128]
    ], 0)
    Wxi = np.concatenate([Wa1[:C], np.zeros((C, 64), np.float32)], 1)
    W22 = np.zeros((128, 128), np.float32)
    W22[:64, :32] = weights["Wa2"]
    W22[64:, 64:] = weights["Wg2"]
    bsel = np.tile(np.eye(RG, dtype=np.float32), (4, 1))
    ident = np.eye(64, dtype=np.float32)
    wts = {
        "We1": weights["We1"], "We2": weights["We2"], "We3": weights["We3"],
        "W96": W96, "Wxi": Wxi, "Wn": weights["Wn"], "W22": W22,
        "Wa3": weights["Wa3"], "Ws": weights["Ws"],
        "Wc1": weights["Wc1"], "Wc2": weights["Wc2"],
        "bsel": bsel, "identb": ident,
    }
    wts = {k: np.ascontiguousarray(v, bfloat16) for k, v in wts.items()}
    wts["identf"] = ident
    for k in ("be1", "be2", "be3", "bn", "ba2", "bg2", "bs", "bc1", "bc2"):
        wts[k] = np.ascontiguousarray(weights[k][:, None], np.float32)
    wts["bhg"] = np.ascontiguousarray(
        np.concatenate([weights["ba1"], weights["bg1"]])[:, None], np.float32)

    in_maps = []
    for core in range(NCORES):
        b = core // 4
        i0 = (core % 4) * RPC
        m = dict(wts)
        rows = np.arange(i0, i0 + RPC)
        jv = jidx[b, rows]                              # [RPC, D]
        # xjf: [64, NG*TG], token t = g*TG + d*RG + r
        ax = x[b][jv]                                   # [RPC, D, C]
        ax = ax.reshape(NG, RG, D, C).transpose(0, 2, 1, 3).reshape(-1, C)
        m["xjf"] = np.ascontiguousarray(ax.T, bfloat16)
        ae = edge_features[b][rows[:, None], jv]        # [RPC, D, E]
        ae = ae.reshape(NG, RG, D, E).transpose(0, 2, 1, 3).reshape(-1, E)
        m["eff"] = np.ascontiguousarray(ae.T, bfloat16)
        m["xrf"] = np.ascontiguousarray(x[b, rows].T, bfloat16)
        # token-major mask [128, NG*NCOL]: token t=c*128+p -> d=t//RG, r=p%RG
        vmask = valid[b, rows].reshape(NG, RG, D)       # [NG, RG, D]
        tt = np.arange(TG)
        dd, rr = tt // RG, tt % RG
        amt = np.where(vmask[:, rr, dd], 0.0, -1e30).astype(np.float32)
        amt = amt.reshape(NG, NCOL, 128).transpose(2, 0, 1).reshape(128, -1)
        m["amt"] = np.ascontiguousarray(amt)
        in_maps.append(m)
    return in_maps


def kernel(**inputs):
    global _NC
    x = np.asarray(inputs["x"], np.float32)
    adjacency = np.asarray(inputs["adjacency"], np.float32)
    edge_features = np.asarray(inputs["edge_features"], np.float32)
    weights = {k: np.asarray(v, np.float32) for k, v in inputs.items()
               if k not in ("x", "adjacency", "edge_features")}
    in_maps = _host_prep(x, adjacency, edge_features, weights)
    if _NC is None:
        _NC = _build_nc()
    res = run_bass_kernel_spmd(_NC, in_maps, list(range(NCORES)))
    out = np.zeros((B, N, O), np.float32)
    for core in range(NCORES):
        b = core // 4
        i0 = (core % 4) * RPC
        out[b, i0:i0 + RPC] = res.results[core]["out"]
    return out


# revision 19
# speedup vs baseline: 18.6158x; 18.6158x over previous
"""Trainium2 Bass kernel for nn_EnhancedGraphConv (gnn_message_passing).

Strategy (8 cores): shard the B*N=1280 graph rows as 160 rows/core (cores
0-3 -> batch 0, 4-7 -> batch 1).  The host converts the dense adjacency to
padded neighbor lists (max degree 51 -> D=52 slots/row) and gathers the
neighbor features (x_j) and edge features directly into feature-major bf16
HBM layouts, so the device streams contiguous tiles: no device-side gather
and no input transposes.  All matmuls run in bf16 (1 cycle/row on PE vs 4
for fp32).  The attention softmax runs token-major: scores land in PSUM as
[128 tokens, 13 cols] whose partition p maps to row p%32, so row max / sum
use small cross-partition DVE ops, and the weighted message reduction is a
PE matmul against a w-scaled block-identity selection matrix.  Exp (softmax)
and Sigmoid (gates) live in different ACT function sets, so all sigmoid-set
work happens in phase 1 and a single batched exp runs in phase 2: exactly
two ACT table loads per invocation.
"""
import numpy as np
from contextlib import ExitStack

import concourse.bass as bass
import concourse.bacc as bacc
import concourse.tile as tile
from concourse import mybir
from concourse.bass_utils import run_bass_kernel_spmd

F32 = mybir.dt.float32
BF16 = mybir.dt.bfloat16
AF = mybir.ActivationFunctionType
OP = mybir.AluOpType

B, N, C, O, E = 2, 640, 64, 64, 18
D = 52            # neighbor slots per row (max degree 51)
RG = 32           # rows per group
NCORES = 8
RPC = (B * N) // NCORES   # 160 rows per core
NG = RPC // RG            # 5 groups
TG = D * RG               # 1664 tokens per group
CH = 416                  # matmul moving chunk
NCH = TG // CH            # 4 chunks per group
NCOL = TG // 128          # 13 token-major columns per group


def _build_nc(debug=False, reps=1):
    nc = bacc.Bacc("TRN2", target_bir_lowering=False)
    t = {}
    winp = [
        ("We1", [E, 64]), ("We2d", [128, 64]), ("We3d", [128, 32]),
        ("Wjj", [64, 128]), ("Wped", [64, 128]), ("Wxi", [64, 128]),
        ("Wn", [64, 64]),
        ("W22", [128, 128]), ("Wa3", [32, 1]), ("Ws", [64, 64]),
        ("Wc1", [128, 64]), ("Wc2", [64, 64]),
        ("bsel", [128, RG]), ("identb", [64, 64]),
    ]
    binp = [
        ("be1d", [128, 1]), ("be2d", [128, 1]), ("be3d", [64, 1]),
        ("bhg", [128, 1]), ("bn", [64, 1]), ("ba2", [32, 1]),
        ("bg2", [64, 1]), ("bs", [64, 1]), ("bc1", [64, 1]), ("bc2", [64, 1]),
    ]
    for name, shape in winp:
        t[name] = nc.dram_tensor(name, shape, BF16, kind="ExternalInput")
    for name, shape in binp:
        t[name] = nc.dram_tensor(name, shape, F32, kind="ExternalInput")
    t["identf"] = nc.dram_tensor("identf", [64, 64], F32, kind="ExternalInput")
    t["xjf"] = nc.dram_tensor("xjf", [64, NG * TG], BF16, kind="ExternalInput")
    t["eff"] = nc.dram_tensor("eff", [E, NG * TG], BF16, kind="ExternalInput")
    t["xrf"] = nc.dram_tensor("xrf", [64, RPC], BF16, kind="ExternalInput")
    t["amt"] = nc.dram_tensor("amt", [128, NG * NCOL], F32, kind="ExternalInput")
    t["out"] = nc.dram_tensor("out", [RPC, O], F32, kind="ExternalOutput")
    if debug:
        t["dbg_h2"] = nc.dram_tensor("dbg_h2", [32, TG], F32, kind="ExternalOutput")
        t["dbg_gtn"] = nc.dram_tensor("dbg_gtn", [64, TG], F32, kind="ExternalOutput")
        t["dbg_wexp"] = nc.dram_tensor("dbg_wexp", [128, NG * NCOL], F32, kind="ExternalOutput")
        t["dbg_msg"] = nc.dram_tensor("dbg_msg", [RG, 64], F32, kind="ExternalOutput")

    with tile.TileContext(nc) as tc, ExitStack() as ctx:
        w = ctx.enter_context(tc.tile_pool(name="w", bufs=1))
        big = ctx.enter_context(tc.tile_pool(name="big", bufs=2))
        sm = ctx.enter_context(tc.tile_pool(name="sm", bufs=2))
        mma = ctx.enter_context(tc.tile_pool(name="mma", bufs=2, space="PSUM"))
        mmb = ctx.enter_context(tc.tile_pool(name="mmb", bufs=2, space="PSUM"))
        pt = ctx.enter_context(tc.tile_pool(name="pt", bufs=1, space="PSUM"))
        pq = ctx.enter_context(tc.tile_pool(name="pq", bufs=1, space="PSUM"))

        # ---- weights / constants -> SBUF
        wt = {}
        for name, shape in winp:
            wt[name] = w.tile(shape, BF16, name=name)
        for name, shape in binp:
            wt[name] = w.tile(shape, F32, name=name)
        for name, _ in winp + binp:
            nc.sync.dma_start(out=wt[name][:], in_=t[name][:])
        identf = w.tile([64, 64], F32, name="identf")
        nc.sync.dma_start(out=identf[:], in_=t["identf"][:])
        xrf = w.tile([64, RPC], BF16, name="xrf")
        nc.sync.dma_start(out=xrf[:], in_=t["xrf"][:])
        amt = w.tile([128, NG * NCOL], F32, name="amt")
        nc.sync.dma_start(out=amt[:], in_=t["amt"][:])

        # ---- prologue: self features for this core's rows
        pself = pq.tile([64, RPC], F32, name="pq")
        nc.tensor.matmul(pself[:], wt["Ws"][:], xrf[:], start=True, stop=True)
        selff = w.tile([64, RPC], BF16, name="selff")
        nc.scalar.activation(selff[:], pself[:], AF.Identity, bias=wt["bs"][:])

        # persistent per-rep state
        smg = w.tile([128, NG * NCOL], F32, name="smg")
        gtm = w.tile([128, NG * NCOL * 64], BF16, name="gtm")

        for rep in range(reps):
            # ================= phase 1: per-group MLPs (sigmoid ACT set)
            # pe-MLP layers run "packed": two 416-token chunks side by side
            # on partition halves, so evictions use all 128 lanes.
            for g in range(NG):
                gsl = slice(g * TG, (g + 1) * TG)
                xj = big.tile([64, TG], BF16, name="xj")
                nc.sync.dma_start(out=xj[:], in_=t["xjf"][:, gsl])
                ef = big.tile([E, TG], BF16, name="ef")
                nc.sync.dma_start(out=ef[:], in_=t["eff"][:, gsl])

                pe1 = big.tile([128, 2 * CH], BF16, name="pe1")
                pe2 = big.tile([128, 2 * CH], BF16, name="pe2")
                pe3 = big.tile([64, 2 * CH], BF16, name="pe3")
                for p in range(2):
                    pc = slice(p * CH, (p + 1) * CH)
                    ca = slice(p * 2 * CH, p * 2 * CH + CH)
                    cb = slice(p * 2 * CH + CH, (p + 1) * 2 * CH)
                    ps = mma.tile([128, CH], F32, name="mma")
                    nc.tensor.matmul(ps[0:64, :], wt["We1"][:], ef[:, ca],
                                     start=True, stop=True,
                                     skip_group_check=True)
                    nc.tensor.matmul(ps[64:128, :], wt["We1"][:], ef[:, cb],
                                     start=True, stop=True,
                                     skip_group_check=True)
                    nc.scalar.activation(pe1[:, pc], ps[:], AF.Relu,
                                         bias=wt["be1d"][:])
                for p in range(2):
                    pc = slice(p * CH, (p + 1) * CH)
                    ps = mma.tile([128, CH], F32, name="mma")
                    nc.tensor.matmul(ps[0:64, :], wt["We2d"][0:64, :],
                                     pe1[0:64, pc], start=True, stop=True,
                                     skip_group_check=True)
                    nc.tensor.matmul(ps[64:128, :], wt["We2d"][64:128, :],
                                     pe1[64:128, pc], start=True, stop=True,
                                     skip_group_check=True)
                    nc.scalar.activation(pe2[:, pc], ps[:], AF.Relu,
                                         bias=wt["be2d"][:])
                for p in range(2):
                    pc = slice(p * CH, (p + 1) * CH)
                    ps = mma.tile([128, CH], F32, name="mma")
                    nc.tensor.matmul(ps[0:32, :], wt["We3d"][0:64, :],
                                     pe2[0:64, pc], start=True, stop=True,
                                     skip_group_check=True)
                    nc.tensor.matmul(ps[32:64, :], wt["We3d"][64:128, :],
                                     pe2[64:128, pc], start=True, stop=True,
                                     skip_group_check=True)
                    nc.vector.tensor_scalar(out=pe3[:, pc], in0=ps[0:64, :],
                                            scalar1=wt["be3d"][:], scalar2=0.0,
                                            op0=OP.add, op1=OP.max)
                # hg = relu(Wjj.xj + Wpe.pe3 + Wxi.xi + bhg), pair-fused evicts
                hg = big.tile([128, TG], BF16, name="hg")
                for p in range(2):
                    psb = mmb.tile([128, 1024], F32, name="mmb")
                    for h in range(2):
                        q = 2 * p + h
                        cols = slice(q * CH, (q + 1) * CH)
                        oc = slice(h * 512, h * 512 + CH)
                        nc.tensor.matmul(psb[:, oc], wt["Wjj"][:],
                                         xj[:, cols], start=True, stop=False)
                        nc.tensor.matmul(psb[:, oc],
                                         wt["Wped"][h * 32:h * 32 + 32, :],
                                         pe3[h * 32:h * 32 + 32, pc],
                                         start=False, stop=False)
                        xi_b = bass.AP(
                            tensor=xrf.tensor,
                            offset=xrf[:, g * RG:(g + 1) * RG].offset,
                            ap=[xrf[:].ap[0], [0, CH // RG], [1, RG]])
                        nc.tensor.matmul(psb[:, oc], wt["Wxi"][:], xi_b,
                                         start=False, stop=True)
                    pr = slice(p * 2 * CH, (p + 1) * 2 * CH)
                    in_v = bass.AP(tensor=psb.tensor, offset=psb[:].offset,
                                   ap=[psb[:].ap[0], [512, 2], [1, CH]])
                    nc.scalar.activation(
                        hg[:, pr].rearrange("p (a b) -> p a b", a=2), in_v,
                        AF.Relu, bias=wt["bhg"][:])
                tn = big.tile([64, TG], BF16, name="tn")
                for p in range(2):
                    psb = mmb.tile([128, 1024], F32, name="mmb")
                    for h in range(2):
                        q = 2 * p + h
                        cols = slice(q * CH, (q + 1) * CH)
                        oc = slice(h * 512, h * 512 + CH)
                        nc.tensor.matmul(psb[0:64, oc], wt["Wn"][:],
                                         xj[:, cols], start=True, stop=True)
                    pr = slice(p * 2 * CH, (p + 1) * 2 * CH)
                    base = psb[0:64, :]
                    in_v = bass.AP(tensor=psb.tensor, offset=base.offset,
                                   ap=[base.ap[0], [512, 2], [1, CH]])
                    nc.vector.tensor_scalar_add(
                        out=tn[:, pr].rearrange("p (a b) -> p a b", a=2),
                        in0=in_v, scalar1=wt["bn"][:])
                h2 = big.tile([32, TG], BF16, name="h2")
                gates = big.tile([64, TG], BF16, name="gates")
                for p in range(2):
                    psb = mmb.tile([128, 1024], F32, name="mmb")
                    for h in range(2):
                        q = 2 * p + h
                        cols = slice(q * CH, (q + 1) * CH)
                        oc = slice(h * 512, h * 512 + CH)
                        nc.tensor.matmul(psb[:, oc], wt["W22"][:],
                                         hg[:, cols], start=True, stop=True)
                    pr = slice(p * 2 * CH, (p + 1) * 2 * CH)
                    bh2 = psb[0:32, :]
                    in_h2 = bass.AP(tensor=psb.tensor, offset=bh2.offset,
                                    ap=[bh2.ap[0], [512, 2], [1, CH]])
                    nc.vector.tensor_scalar(
                        out=h2[:, pr].rearrange("p (a b) -> p a b", a=2),
                        in0=in_h2, scalar1=wt["ba2"][:], scalar2=0.0,
                        op0=OP.add, op1=OP.max)
                    bga = psb[64:128, :]
                    in_ga = bass.AP(tensor=psb.tensor, offset=bga.offset,
                                    ap=[bga.ap[0], [512, 2], [1, CH]])
                    nc.scalar.activation(
                        gates[:, pr].rearrange("p (a b) -> p a b", a=2),
                        in_ga, AF.Sigmoid, bias=wt["bg2"][:])
                gtn = big.tile([64, TG], BF16, name="gtn")
                nc.vector.tensor_tensor(out=gtn[:], in0=gates[:], in1=tn[:],
                                        op=OP.mult)
                # scores -> masked, token-major [128, NCOL]
                psc = pq.tile([128, NCOL], F32, name="pq")
                for c in range(NCOL):
                    nc.tensor.matmul(psc[:, c:c + 1],
                                     h2[:, c * 128:(c + 1) * 128],
                                     wt["Wa3"][:], start=True, stop=True)
                csl = slice(g * NCOL, (g + 1) * NCOL)
                nc.vector.tensor_tensor(out=smg[:, csl], in0=psc[:],
                                        in1=amt[:, csl], op=OP.add)
                # transpose gtn to token-major chunks [128, 64], one evict
                ptm = pt.tile([128, NCOL * 64], BF16, name="pt")
                for c in range(NCOL):
                    nc.tensor.transpose(ptm[:, c * 64:(c + 1) * 64],
                                        gtn[:, c * 128:(c + 1) * 128],
                                        wt["identb"][:])
                k = g * NCOL * 64
                nc.scalar.activation(gtm[:, k:k + NCOL * 64], ptm[:], AF.Copy)
                if debug and g == 0 and rep == 0:
                    nc.sync.dma_start(out=t["dbg_h2"][:], in_=h2[:])
                    nc.sync.dma_start(out=t["dbg_gtn"][:], in_=gtn[:])

            # ================= phase 2: batched masked softmax pieces (exp set)
            # cross-partition folds route one operand through PSUM (walrus
            # forbids SB+SB TensorTensor with mismatched base partitions)
            rmax = sm.tile([128, NG], F32, name="rmax")
            nc.vector.tensor_reduce(
                out=rmax[:], in_=smg[:].rearrange("p (g c) -> p g c", g=NG),
                axis=mybir.AxisListType.X, op=OP.max)
            prmax = pq.tile([128, NG], F32, name="pq")
            nc.vector.tensor_copy(out=prmax[:], in_=rmax[:])
            m2 = sm.tile([64, NG], F32, name="m2")
            nc.vector.tensor_tensor(out=m2[:], in0=prmax[64:128, :],
                                    in1=rmax[0:64, :], op=OP.max)
            pm2 = pq.tile([64, NG], F32, name="pq")
            nc.vector.tensor_copy(out=pm2[:], in_=m2[:])
            nm = sm.tile([32, NG], F32, name="nm")
            nc.vector.tensor_tensor(out=nm[:], in0=pm2[32:64, :],
                                    in1=m2[0:32, :], op=OP.max)
            nc.vector.tensor_scalar_mul(out=nm[:], in0=nm[:], scalar1=-1.0)
            pnm = pq.tile([32, NG], F32, name="pq")
            nc.vector.tensor_copy(out=pnm[:], in_=nm[:])
            nm128 = sm.tile([128, NG], F32, name="nm128")
            for blk in range(4):
                nc.vector.tensor_copy(out=nm128[blk * 32:(blk + 1) * 32, :],
                                      in_=pnm[:])
            wsub = sm.tile([128, NG * NCOL], F32, name="wsub")
            nm_b = bass.AP(tensor=nm128.tensor, offset=nm128[:].offset,
                           ap=[nm128[:].ap[0], [1, NG], [0, NCOL]])
            nc.vector.tensor_tensor(out=wsub[:], in0=smg[:], in1=nm_b,
                                    op=OP.add)
            wexp = sm.tile([128, NG * NCOL], BF16, name="wexp")
            nc.scalar.activation(wexp[:], wsub[:], AF.Exp)
            if debug and rep == 0:
                nc.sync.dma_start(out=t["dbg_wexp"][:], in_=wexp[:])
            # Z per (row, group) via PE: block-identity contracts partitions
            pZ = pq.tile([32, NG * NCOL], F32, name="pq")
            nc.tensor.matmul(pZ[:], wt["bsel"][:], wexp[:],
                             start=True, stop=True)
            invz = sm.tile([32, NG], F32, name="invz")
            nc.vector.tensor_reduce(
                out=invz[:], in_=pZ[:].rearrange("p (g c) -> p g c", g=NG),
                axis=mybir.AxisListType.X, op=OP.add)
            nc.vector.tensor_scalar_add(out=invz[:], in0=invz[:], scalar1=1e-30)
            nc.vector.reciprocal(out=invz[:], in_=invz[:])

            # ================= phase 3: weighted reduce + output MLP (exp set)
            for g in range(NG):
                wsel = sm.tile([128, NCOL * RG], BF16, name="wsel")
                bsel_b = bass.AP(tensor=wt["bsel"].tensor,
                                 offset=wt["bsel"][:].offset,
                                 ap=[wt["bsel"][:].ap[0], [0, NCOL], [1, RG]])
                we_b = bass.AP(tensor=wexp.tensor,
                               offset=wexp[:, g * NCOL:(g + 1) * NCOL].offset,
                               ap=[wexp[:].ap[0], [1, NCOL], [0, RG]])
                nc.vector.tensor_tensor(out=wsel[:], in0=bsel_b, in1=we_b,
                                        op=OP.mult)
                pmsg = pq.tile([32, 64], F32, name="pq")
                for c in range(NCOL):
                    k = (g * NCOL + c) * 64
                    nc.tensor.matmul(pmsg[:], wsel[:, c * RG:(c + 1) * RG],
                                     gtm[:, k:k + 64],
                                     start=(c == 0), stop=(c == NCOL - 1))
                msg = sm.tile([32, 64], BF16, name="msg")
                nc.vector.tensor_scalar_mul(out=msg[:], in0=pmsg[:],
                                            scalar1=invz[:, g:g + 1])
                if debug and g == 0 and rep == 0:
                    nc.sync.dma_start(out=t["dbg_msg"][:], in_=msg[:])
                pmt = pq.tile([64, RG], BF16, name="pq")
                nc.tensor.transpose(pmt[:], msg[:], wt["identb"][:32, :32])
                comb = sm.tile([128, RG], BF16, name="comb")
                nc.scalar.activation(comb[:64, :],
                                     selff[:, g * RG:(g + 1) * RG], AF.Copy)
                nc.scalar.activation(comb[64:128, :], pmt[:], AF.Copy)
                pc1 = pq.tile([64, RG], F32, name="pq")
                nc.tensor.matmul(pc1[:], wt["Wc1"][:], comb[:],
                                 start=True, stop=True)
                c1 = sm.tile([64, RG], BF16, name="c1")
                nc.scalar.activation(c1[:], pc1[:], AF.Relu, bias=wt["bc1"][:])
                pc2 = pq.tile([64, RG], F32, name="pq")
                nc.tensor.matmul(pc2[:], wt["Wc2"][:], c1[:],
                                 start=True, stop=True)
                ofm = sm.tile([64, RG], F32, name="ofm")
                nc.scalar.activation(ofm[:], pc2[:], AF.Identity,
                                     bias=wt["bc2"][:])
                por = pq.tile([RG, 64], F32, name="pq")
                nc.tensor.transpose(por[:], ofm[:], identf[:])
                orow = sm.tile([RG, 64], F32, name="orow")
                nc.vector.tensor_copy(out=orow[:], in_=por[:])
                nc.sync.dma_start(out=t["out"][g * RG:(g + 1) * RG, :],
                                  in_=orow[:])
    nc.compile()
    return nc


_NC = None


def _host_prep(x, adjacency, edge_features, weights):
    """Build per-core input maps (bf16 feature-major gathered layouts)."""
    from ml_dtypes import bfloat16
    adj = adjacency > 0
    order = np.argsort(~adj, axis=-1, kind="stable")   # [B, N, N]
    deg = adj.sum(-1)                                  # [B, N]
    assert deg.max() <= D, f"degree {deg.max()} exceeds {D} slots"
    jidx = order[:, :, :D].astype(np.int64)            # [B, N, D]
    slot = np.arange(D)[None, None, :]
    valid = slot < deg[:, :, None]                     # [B, N, D]
    jidx = np.where(valid, jidx, 0)

    C2 = 2 * C
    Wa1, Wg1 = weights["Wa1"], weights["Wg1"]
    Wjj = np.concatenate([Wa1[C:C2], Wg1[:C]], 1)      # [64, 128]
    Wpe = np.concatenate([Wa1[C2:], Wg1[C:]], 1)       # [32, 128]
    Wxi = np.concatenate([Wa1[:C], np.zeros((C, 64), np.float32)], 1)
    W22 = np.zeros((128, 128), np.float32)
    W22[:64, :32] = weights["Wa2"]
    W22[64:, 64:] = weights["Wg2"]
    bsel = np.tile(np.eye(RG, dtype=np.float32), (4, 1))
    ident = np.eye(64, dtype=np.float32)
    dbl = lambda a: np.concatenate([a, a], 0)
    wts = {
        "We1": weights["We1"], "We2d": dbl(weights["We2"]),
        "We3d": dbl(weights["We3"]),
        "Wjj": Wjj, "Wped": dbl(Wpe), "Wxi": Wxi,
        "Wn": weights["Wn"], "W22": W22,
        "Wa3": weights["Wa3"], "Ws": weights["Ws"],
        "Wc1": weights["Wc1"], "Wc2": weights["Wc2"],
        "bsel": bsel, "identb": ident,
    }
    wts = {k: np.ascontiguousarray(v, bfloat16) for k, v in wts.items()}
    wts["identf"] = ident
    for k in ("bn", "ba2", "bg2", "bs", "bc1", "bc2"):
        wts[k] = np.ascontiguousarray(weights[k][:, None], np.float32)
    wts["be1d"] = np.ascontiguousarray(dbl(weights["be1"])[:, None], np.float32)
    wts["be2d"] = np.ascontiguousarray(dbl(weights["be2"])[:, None], np.float32)
    wts["be3d"] = np.ascontiguousarray(dbl(weights["be3"])[:, None], np.float32)
    wts["bhg"] = np.ascontiguousarray(
        np.concatenate([weights["ba1"], weights["bg1"]])[:, None], np.float32)

    in_maps = []
    for core in range(NCORES):
        b = core // 4
        i0 = (core % 4) * RPC
        m = dict(wts)
        rows = np.arange(i0, i0 + RPC)
        jv = jidx[b, rows]                              # [RPC, D]
        # xjf: [64, NG*TG], token t = g*TG + d*RG + r
        ax = x[b][jv]                                   # [RPC, D, C]
        ax = ax.reshape(NG, RG, D, C).transpose(0, 2, 1, 3).reshape(-1, C)
        m["xjf"] = np.ascontiguousarray(ax.T, bfloat16)
        ae = edge_features[b][rows[:, None], jv]        # [RPC, D, E]
        ae = ae.reshape(NG, RG, D, E).transpose(0, 2, 1, 3).reshape(-1, E)
        m["eff"] = np.ascontiguousarray(ae.T, bfloat16)
        m["xrf"] = np.ascontiguousarray(x[b, rows].T, bfloat16)
        # token-major mask [128, NG*NCOL]: token t=c*128+p -> d=t//RG, r=p%RG
        vmask = valid[b, rows].reshape(NG, RG, D)       # [NG, RG, D]
        tt = np.arange(TG)
        dd, rr = tt // RG, tt % RG
        amt = np.where(vmask[:, rr, dd], 0.0, -1e30).astype(np.float32)
        amt = amt.reshape(NG, NCOL, 128).transpose(2, 0, 1).reshape(128, -1)
        m["amt"] = np.ascontiguousarray(amt)
        in_maps.append(m)
    return in_maps


def kernel(**inputs):
    global _NC
    x = np.asarray(inputs["x"], np.float32)
    adjacency = np.asarray(inputs["adjacency"], np.float32)
    edge_features = np.asarray(inputs["edge_features"], np.float32)
    weights = {k: np.asarray(v, np.float32) for k, v in inputs.items()
               if k not in ("x", "adjacency", "edge_features")}
    in_maps = _host_prep(x, adjacency, edge_features, weights)
    if _NC is None:
        _NC = _build_nc()
    res = run_bass_kernel_spmd(_NC, in_maps, list(range(NCORES)))
    out = np.zeros((B, N, O), np.float32)
    for core in range(NCORES):
        b = core // 4
        i0 = (core % 4) * RPC
        out[b, i0:i0 + RPC] = res.results[core]["out"]
    return out
